# revision 10
# baseline (speedup 1.0000x reference)
"""GRU cell kernel for Trainium2, 8-core data-parallel, single dispatch.

Strategy
--------
Data-parallel on batch across 8 cores; each core handles 2048 rows in ONE
NEFF dispatch, processed as two in-kernel chunks of 1024 columns (SBUF
capacity).  All on-chip compute is in transposed space ([hidden, batch])
so matmul contractions land on SBUF partitions with no on-device
transposes:

    r^T = sigmoid(W_r @ x^T + U_r @ h^T + b_r)
    u^T = sigmoid(W_u @ x^T + U_u @ h^T + b_u)
    c^T = tanh   (W   @ x^T + U  @ (h.r)^T + b_c)
    o^T = h^T + u^T * (c^T - h^T)

Matmuls run in bf16.  v1 profiling showed the kernel was gated by load
DMAs: 409 per-tile descriptors (32-256KB each) serialized on the Sync
engine's single HWDGE ring at ~600ns/descriptor, leaving LDWEIGHTS
waiting 0.5-0.8us on weight arrival between every matmul pair (340ns
observed MM spacing vs the 226ns streaming floor).  Sub-64KB DMAs are
descriptor-dominated (~52GB/s); >=1MB DMAs hit 341-425GB/s.

So v2 host-prepacks everything into big [128, F] blocks and loads with a
handful of ~2MB DMAs split across BOTH HWDGE rings (weights on the
qAct ring via nc.scalar, x/h on the qSP ring via nc.sync) so weight
prefetch never queues behind input streaming.  All tiles are fully
SBUF-resident with no DMA-written slot ever recycled (this toolchain's
DMA descriptors carry exactly ONE sync wait, so a load needing a WAR
wait on top of its queue wait fails codegen).  Stores ride SWDGE
(gpsimd) queues with their single RAW wait.

The R phase is software-pipelined (x-parts of j+2 interleave with
h-parts of j) so the PE has x-side work while U_r/h^T are still in
flight at kernel start.  U/C/OUT are fused per j: u_j and c_j feed the
output combine immediately, so only r/u/c transients of 2 tiles each are
live and everything fits in ~196KB/partition of SBUF.
"""

import sys

sys.path.insert(0, "/opt/trn_rl_repo")

import numpy as np
import ml_dtypes
from contextlib import ExitStack

import concourse.bass as bass
import concourse.bacc as bacc
import concourse.mybir as mybir
from concourse import tile
from concourse.bass_utils import run_bass_kernel_spmd

BF16 = mybir.dt.bfloat16
F32 = mybir.dt.float32
AF = mybir.ActivationFunctionType

N_CORES = 8
B = 16384
D = 1024  # IN == H
B_CORE = B // N_CORES  # 2048 rows per core
N_CHUNKS = 2
CW = B_CORE // N_CHUNKS  # 1024 batch columns per chunk
BW = 512  # matmul moving width (one fp32 PSUM bank)
NH = D // 128  # 8 hidden row-tiles
NK = D // 128  # 8 contraction tiles
NB = CW // BW  # 2 PSUM banks per chunk row


def build_nc():
    """Build the SPMD per-core Bass program.

    DRAM inputs (all host-prepacked, partition-major):
      wts  [6, 128, NH*NK*128] bf16 : per mat m, wts[m][p, (j*NK+k)*128+c]
                                      = M_m.T[k*128+p, j*128+c]
                                      (mats: 0=W_r 1=U_r 2=W_u 3=U_u 4=W 5=U)
      xt   [N_CHUNKS, 128, NK*CW]   : xt[ch][p, k*CW+c] = x.T[k*128+p, ch*CW+c]
      ht   [N_CHUNKS, 128, NK*CW]   : same for h_prev
      bias [128, 3*NH] f32          : bias[p, g*NH+j] = b_g[j*128+p]
    Output:
      out  [D, B_CORE] f32          : out[d, b] = o.T[d, b]
    """
    nc = bacc.Bacc("TRN2", target_bir_lowering=False)
    wts = nc.dram_tensor("wts", [6, 128, NH * NK * 128], BF16, kind="ExternalInput")
    xt = nc.dram_tensor("xt", [N_CHUNKS, 128, NK * CW], BF16, kind="ExternalInput")
    ht = nc.dram_tensor("ht", [N_CHUNKS, 128, NK * CW], BF16, kind="ExternalInput")
    bias = nc.dram_tensor("bias", [128, 3 * NH], F32, kind="ExternalInput")
    out = nc.dram_tensor("out", [D, B_CORE], F32, kind="ExternalOutput")

    with tile.TileContext(nc) as tc, ExitStack() as ctx:
        wp = ctx.enter_context(tc.tile_pool(name="wp", bufs=6))
        xp = ctx.enter_context(tc.tile_pool(name="xp", bufs=N_CHUNKS))
        hp = ctx.enter_context(tc.tile_pool(name="hp", bufs=N_CHUNKS))
        hrp = ctx.enter_context(tc.tile_pool(name="hrp", bufs=NH))
        rp = ctx.enter_context(tc.tile_pool(name="rp", bufs=2))
        up = ctx.enter_context(tc.tile_pool(name="up", bufs=2))
        cp = ctx.enter_context(tc.tile_pool(name="cp", bufs=2))
        op = ctx.enter_context(tc.tile_pool(name="op", bufs=2))
        bp = ctx.enter_context(tc.tile_pool(name="bp", bufs=1))
        pp = ctx.enter_context(tc.tile_pool(name="pp", bufs=8, space="PSUM"))

        gp = ctx.enter_context(tc.tile_pool(name="gp", bufs=1))

        # ---- PE warm-up: ~36 dummy matmuls on a memset tile keep the PE
        # busy through the HAM SHORT window while the first loads land, so
        # real matmuls start at K=8/8 (saves the ~13us cold-throttle tax).
        gtile = gp.tile([128, BW], BF16, name="gtile")
        nc.vector.memset(gtile, 0.0)
        psd = pp.tile([128, BW], F32, name="ps")
        N_WARM = 36
        for i in range(N_WARM):
            nc.tensor.matmul(
                psd, gtile[:, :128], gtile, start=(i == 0), stop=(i == N_WARM - 1)
            )

        # ---- loads: split across the two HWDGE rings in demand order.
        # First-chunk tensors are j/k-sliced (~256KB pieces) so the first
        # matmul's dependency set is ~0.5MB, not 4MB.
        wtiles = [
            wp.tile([128, NH * NK * 128], BF16, name="wtile") for _ in range(6)
        ]
        xts, hts = [], []
        for ch in range(N_CHUNKS):
            xts.append(xp.tile([128, NK * CW], BF16, name="xtile"))
            hts.append(hp.tile([128, NK * CW], BF16, name="htile"))
        btile = bp.tile([128, 3 * NH], F32, name="btile")

        JW = NK * 128  # 1024 cols per j-block of a weight tile

        def ld_w(eng, m, j):
            eng.dma_start(
                wtiles[m][:, j * JW : (j + 1) * JW], wts[m, :, j * JW : (j + 1) * JW]
            )

        def ld_bt(eng, tile_, dram, ch, k):
            eng.dma_start(
                tile_[:, k * CW : (k + 1) * CW], dram[ch, :, k * CW : (k + 1) * CW]
            )

        A, Bq = nc.scalar, nc.sync
        # ring A (qAct): W_r j-sliced + U_r j-sliced interleaved, then the
        # three late-phase mats whole; x0's first ks ride here too so the
        # j0 x-part can start ~10us in.
        ld_w(A, 0, 0)
        for k in range(4):
            ld_bt(A, xts[0], xt, 0, k)
        for j in (1, 2, 3, 4):
            ld_w(A, 0, j)
        ld_w(A, 1, 0)
        ld_w(A, 0, 5)
        ld_w(A, 1, 1)
        ld_w(A, 0, 6)
        ld_w(A, 1, 2)
        ld_w(A, 0, 7)
        for j in (3, 4, 5, 6, 7):
            ld_w(A, 1, j)
        for m in (2, 3, 4, 5):
            A.dma_start(wtiles[m], wts[m, :, :])
        # ring B (qSP): bias, rest of x0, h0 k-sliced, chunk-1 whole.
        Bq.dma_start(btile, bias[:, :])
        for k in range(4, NK):
            ld_bt(Bq, xts[0], xt, 0, k)
        for k in range(NK):
            ld_bt(Bq, hts[0], ht, 0, k)
        Bq.dma_start(xts[1], xt[1, :, :])
        Bq.dma_start(hts[1], ht[1, :, :])

        def w_ap(m, j, k):
            return wtiles[m][:, (j * NK + k) * 128 : (j * NK + k + 1) * 128]

        def mm_half(ps, m, j, mov, start, stop, korder=None):
            """One mat's contraction into NB psum banks (bank innermost)."""
            for i, k in enumerate(korder or range(NK)):
                lhsT = w_ap(m, j, k)
                for b in range(NB):
                    nc.tensor.matmul(
                        ps[b],
                        lhsT,
                        mov[:, k * CW + b * BW : k * CW + (b + 1) * BW],
                        start=(start and i == 0),
                        stop=(stop and i == NK - 1),
                    )

        def mm_half_t(ps, m, j, movs, start, stop):
            """Same but moving operand is per-k tiles (hr)."""
            for k in range(NK):
                lhsT = w_ap(m, j, k)
                for b in range(NB):
                    nc.tensor.matmul(
                        ps[b],
                        lhsT,
                        movs[k][:, b * BW : (b + 1) * BW],
                        start=(start and k == 0),
                        stop=(stop and k == NK - 1),
                    )

        def activate(dst, ps, fn, bcol):
            for b in range(NB):
                nc.scalar.activation(
                    dst[:, b * BW : (b + 1) * BW], ps[b], fn,
                    bias=btile[:, bcol : bcol + 1],
                )

        for ch in range(N_CHUNKS):
            xc, hc = xts[ch], hts[ch]

            # ---- R phase, software-pipelined depth 2: the x-parts of
            # j+1/j+2 give the PE work while h^T (and U_r) are still in
            # flight at kernel start.  Live PSUM groups <= 3 (6 banks).
            hrs = []

            def finish_r(j, ps):
                mm_half(ps, 1, j, hc, start=False, stop=True)
                rtile = rp.tile([128, CW], BF16, name="rtile")
                activate(rtile, ps, AF.Sigmoid, j)
                hrtile = hrp.tile([128, CW], BF16, name="hrtile")
                nc.vector.tensor_mul(
                    hrtile, hc[:, j * CW : (j + 1) * CW], rtile
                )
                hrs.append(hrtile)

            ps_list = []
            for j in range(NH):
                ps = [pp.tile([128, BW], F32, name="ps") for _ in range(NB)]
                # first real matmuls consume x0's k-slices in DMA-arrival
                # order (rings A/B interleave) instead of stalling on k=0
                ko = [4, 0, 5, 1, 6, 2, 7, 3] if (ch == 0 and j == 0) else None
                mm_half(ps, 0, j, xc, start=True, stop=False, korder=ko)
                ps_list.append(ps)
                if j >= 2:
                    finish_r(j - 2, ps_list[j - 2])
            finish_r(NH - 2, ps_list[NH - 2])
            finish_r(NH - 1, ps_list[NH - 1])

            # ---- U + C + OUT fused per j ----
            for j in range(NH):
                psu = [pp.tile([128, BW], F32, name="ps") for _ in range(NB)]
                mm_half(psu, 2, j, xc, start=True, stop=False)
                mm_half(psu, 3, j, hc, start=False, stop=True)
                util = up.tile([128, CW], BF16, name="utile")
                activate(util, psu, AF.Sigmoid, NH + j)

                psc = [pp.tile([128, BW], F32, name="ps") for _ in range(NB)]
                mm_half(psc, 4, j, xc, start=True, stop=False)
                mm_half_t(psc, 5, j, hrs, start=False, stop=True)
                ctile = cp.tile([128, CW], BF16, name="ctile")
                activate(ctile, psc, AF.Tanh, 2 * NH + j)

                # per-bank combine + store: bank 0's chain overlaps bank 1's
                # activation, shrinking the end-of-kernel tail
                hj = hc[:, j * CW : (j + 1) * CW]
                t = op.tile([128, CW], F32, name="ttile")
                for b in range(NB):
                    sl = slice(b * BW, (b + 1) * BW)
                    nc.vector.tensor_sub(t[:, sl], ctile[:, sl], hj[:, sl])
                    nc.vector.tensor_mul(t[:, sl], util[:, sl], t[:, sl])
                    nc.vector.tensor_add(t[:, sl], t[:, sl], hj[:, sl])
                    nc.gpsimd.dma_start(
                        out[
                            j * 128 : (j + 1) * 128,
                            ch * CW + b * BW : ch * CW + (b + 1) * BW,
                        ],
                        t[:, sl],
                    )

    nc.compile()
    return nc


def pack_inputs(inputs):
    """Host-side shard + transpose + pack + cast. Per-core input maps."""
    x = np.asarray(inputs["x_t"], np.float32)
    h = np.asarray(inputs["h_prev"], np.float32)

    mats = [inputs["W_r"], inputs["U_r"], inputs["W_u"], inputs["U_u"],
            inputs["W"], inputs["U"]]
    wts = np.empty((6, 128, NH * NK * 128), ml_dtypes.bfloat16)
    for i, m in enumerate(mats):
        mt = np.asarray(m, np.float32).T.astype(ml_dtypes.bfloat16)  # [in, out]
        # [k,p,j,c] -> [p,j,k,c]
        wts[i] = (
            mt.reshape(NK, 128, NH, 128)
            .transpose(1, 2, 0, 3)
            .reshape(128, NH * NK * 128)
        )

    b_r = np.asarray(inputs["b_Wr"], np.float32) + np.asarray(inputs["b_Ur"], np.float32)
    b_u = np.asarray(inputs["b_Wu"], np.float32) + np.asarray(inputs["b_Uu"], np.float32)
    b_c = np.asarray(inputs["b_W"], np.float32) + np.asarray(inputs["b_U"], np.float32)
    bias = np.concatenate(
        [bb.reshape(NH, 128).T for bb in (b_r, b_u, b_c)], axis=1
    ).astype(np.float32)  # [128, 3*NH]

    def pack_bt(a_rows):  # [B_CORE, D] f32 -> [N_CHUNKS, 128, NK*CW] bf16
        at = np.ascontiguousarray(a_rows.T).astype(ml_dtypes.bfloat16)  # [D, B_CORE]
        o = np.empty((N_CHUNKS, 128, NK * CW), ml_dtypes.bfloat16)
        for ch in range(N_CHUNKS):
            o[ch] = (
                at[:, ch * CW : (ch + 1) * CW]
                .reshape(NK, 128, CW)
                .transpose(1, 0, 2)
                .reshape(128, NK * CW)
            )
        return o

    in_maps = []
    for c in range(N_CORES):
        rows = slice(c * B_CORE, (c + 1) * B_CORE)
        in_maps.append(
            {
                "xt": pack_bt(x[rows]),
                "ht": pack_bt(h[rows]),
                "wts": wts,
                "bias": bias,
            }
        )
    return in_maps


_NC_CACHE = {}


def _get_nc():
    if "nc" not in _NC_CACHE:
        _NC_CACHE["nc"] = build_nc()
    return _NC_CACHE["nc"]


def _run(inputs, **spmd_kwargs):
    nc = _get_nc()
    in_maps = pack_inputs(inputs)
    res = run_bass_kernel_spmd(nc, in_maps, list(range(N_CORES)), **spmd_kwargs)
    out = np.empty((B, D), np.float32)
    for c in range(N_CORES):
        out[c * B_CORE : (c + 1) * B_CORE, :] = res.results[c]["out"].T
    return out, [res]


def kernel(**inputs):
    out, _ = _run(inputs)
    return out


# revision 11
# speedup vs baseline: 1.2033x; 1.2033x over previous
"""GRU cell kernel for Trainium2, 8-core data-parallel, single dispatch.

Strategy
--------
Data-parallel on batch across 8 cores; each core handles 2048 rows in ONE
NEFF dispatch, processed as two in-kernel chunks of 1024 columns (SBUF
capacity).  All on-chip compute is in transposed space ([hidden, batch])
so matmul contractions land on SBUF partitions with no on-device
transposes:

    r^T = sigmoid(W_r @ x^T + U_r @ h^T + b_r)
    u^T = sigmoid(W_u @ x^T + U_u @ h^T + b_u)
    c^T = tanh   (W   @ x^T + U  @ (h.r)^T + b_c)
    o^T = h^T + u^T * (c^T - h^T)

Matmuls run in bf16.  v1 profiling showed the kernel was gated by load
DMAs: 409 per-tile descriptors (32-256KB each) serialized on the Sync
engine's single HWDGE ring at ~600ns/descriptor, leaving LDWEIGHTS
waiting 0.5-0.8us on weight arrival between every matmul pair (340ns
observed MM spacing vs the 226ns streaming floor).  Sub-64KB DMAs are
descriptor-dominated (~52GB/s); >=1MB DMAs hit 341-425GB/s.

So v2 host-prepacks everything into big [128, F] blocks and loads with a
handful of ~2MB DMAs split across BOTH HWDGE rings (weights on the
qAct ring via nc.scalar, x/h on the qSP ring via nc.sync) so weight
prefetch never queues behind input streaming.  All tiles are fully
SBUF-resident with no DMA-written slot ever recycled (this toolchain's
DMA descriptors carry exactly ONE sync wait, so a load needing a WAR
wait on top of its queue wait fails codegen).  Stores ride SWDGE
(gpsimd) queues with their single RAW wait.

The R phase is software-pipelined (x-parts of j+2 interleave with
h-parts of j) so the PE has x-side work while U_r/h^T are still in
flight at kernel start.  U/C/OUT are fused per j: u_j and c_j feed the
output combine immediately, so only r/u/c transients of 2 tiles each are
live and everything fits in ~196KB/partition of SBUF.
"""

import sys

sys.path.insert(0, "/opt/trn_rl_repo")

import numpy as np
import ml_dtypes
from contextlib import ExitStack

import concourse.bass as bass
import concourse.bacc as bacc
import concourse.mybir as mybir
from concourse import tile
from concourse.bass_utils import run_bass_kernel_spmd

BF16 = mybir.dt.bfloat16
F32 = mybir.dt.float32
AF = mybir.ActivationFunctionType

N_CORES = 8
B = 16384
D = 1024  # IN == H
B_CORE = B // N_CORES  # 2048 rows per core
N_CHUNKS = 2
CW = B_CORE // N_CHUNKS  # 1024 batch columns per chunk
BW = 512  # matmul moving width (one fp32 PSUM bank)
NH = D // 128  # 8 hidden row-tiles
NK = D // 128  # 8 contraction tiles
NB = CW // BW  # 2 PSUM banks per chunk row


def build_nc():
    """Build the SPMD per-core Bass program.

    DRAM inputs (all host-prepacked, partition-major):
      wts  [6, 128, NH*NK*128] bf16 : per mat m, wts[m][p, (j*NK+k)*128+c]
                                      = M_m.T[k*128+p, j*128+c]
                                      (mats: 0=W_r 1=U_r 2=W_u 3=U_u 4=W 5=U)
      xt   [N_CHUNKS, 128, NK*CW]   : xt[ch][p, k*CW+c] = x.T[k*128+p, ch*CW+c]
      ht   [N_CHUNKS, 128, NK*CW]   : same for h_prev
      bias [128, 3*NH] f32          : bias[p, g*NH+j] = b_g[j*128+p]
    Output:
      out  [D, B_CORE] f32          : out[d, b] = o.T[d, b]
    """
    nc = bacc.Bacc("TRN2", target_bir_lowering=False)
    wts = nc.dram_tensor("wts", [6, 128, NH * NK * 128], BF16, kind="ExternalInput")
    xt = nc.dram_tensor("xt", [N_CHUNKS, 128, NK * CW], BF16, kind="ExternalInput")
    ht = nc.dram_tensor("ht", [N_CHUNKS, 128, NK * CW], BF16, kind="ExternalInput")
    bias = nc.dram_tensor("bias", [128, 3 * NH], F32, kind="ExternalInput")
    out = nc.dram_tensor("out", [D, B_CORE], F32, kind="ExternalOutput")

    with tile.TileContext(nc) as tc, ExitStack() as ctx:
        wp = ctx.enter_context(tc.tile_pool(name="wp", bufs=6))
        xp = ctx.enter_context(tc.tile_pool(name="xp", bufs=N_CHUNKS))
        hp = ctx.enter_context(tc.tile_pool(name="hp", bufs=N_CHUNKS))
        hrp = ctx.enter_context(tc.tile_pool(name="hrp", bufs=NH))
        rp = ctx.enter_context(tc.tile_pool(name="rp", bufs=2))
        up = ctx.enter_context(tc.tile_pool(name="up", bufs=2))
        cp = ctx.enter_context(tc.tile_pool(name="cp", bufs=2))
        op = ctx.enter_context(tc.tile_pool(name="op", bufs=2))
        bp = ctx.enter_context(tc.tile_pool(name="bp", bufs=1))
        pp = ctx.enter_context(tc.tile_pool(name="pp", bufs=8, space="PSUM"))

        # NOTE: no PE warm-up matmuls here.  A warm-up group into one PSUM
        # slot shifts the pool ring so every real accumulation group's bank
        # pair lands misaligned (odd,even) — measured +43ns on EVERY matmul
        # (259ns vs the 216ns streaming floor).  Same-bank warm-up MMs also
        # serialize at ~512ns and delay the first real matmul past the
        # point where its DMAs have landed.

        # ---- loads: split across the two HWDGE rings in demand order.
        # First-chunk tensors are j/k-sliced (~256KB pieces) so the first
        # matmul's dependency set is ~0.5MB, not 4MB.
        wtiles = [
            wp.tile([128, NH * NK * 128], BF16, name="wtile") for _ in range(6)
        ]
        xts, hts = [], []
        for ch in range(N_CHUNKS):
            xts.append(xp.tile([128, NK * CW], BF16, name="xtile"))
            hts.append(hp.tile([128, NK * CW], BF16, name="htile"))
        btile = bp.tile([128, 3 * NH], F32, name="btile")

        JW = NK * 128  # 1024 cols per j-block of a weight tile

        def ld_w(eng, m, j):
            eng.dma_start(
                wtiles[m][:, j * JW : (j + 1) * JW], wts[m, :, j * JW : (j + 1) * JW]
            )

        def ld_bt(eng, tile_, dram, ch, k):
            eng.dma_start(
                tile_[:, k * CW : (k + 1) * CW], dram[ch, :, k * CW : (k + 1) * CW]
            )

        A, Bq = nc.scalar, nc.sync
        # ring A (qAct): W_r j-sliced + U_r j-sliced interleaved, then the
        # three late-phase mats whole; x0's first ks ride here too so the
        # j0 x-part can start ~10us in.
        ld_w(A, 0, 0)
        for k in range(4):
            ld_bt(A, xts[0], xt, 0, k)
        for j in (1, 2, 3, 4):
            ld_w(A, 0, j)
        ld_w(A, 1, 0)
        ld_w(A, 0, 5)
        ld_w(A, 1, 1)
        ld_w(A, 0, 6)
        ld_w(A, 1, 2)
        ld_w(A, 0, 7)
        for j in (3, 4, 5, 6, 7):
            ld_w(A, 1, j)
        for m in (2, 3, 4, 5):
            A.dma_start(wtiles[m], wts[m, :, :])
        # ring B (qSP): bias, rest of x0, h0 k-sliced, chunk-1 whole.
        Bq.dma_start(btile, bias[:, :])
        for k in range(4, NK):
            ld_bt(Bq, xts[0], xt, 0, k)
        for k in range(NK):
            ld_bt(Bq, hts[0], ht, 0, k)
        Bq.dma_start(xts[1], xt[1, :, :])
        Bq.dma_start(hts[1], ht[1, :, :])

        def w_ap(m, j, k):
            return wtiles[m][:, (j * NK + k) * 128 : (j * NK + k + 1) * 128]

        def mm_half(ps, m, j, mov, start, stop, korder=None):
            """One mat's contraction into NB psum banks (bank innermost)."""
            for i, k in enumerate(korder or range(NK)):
                lhsT = w_ap(m, j, k)
                for b in range(NB):
                    nc.tensor.matmul(
                        ps[b],
                        lhsT,
                        mov[:, k * CW + b * BW : k * CW + (b + 1) * BW],
                        start=(start and i == 0),
                        stop=(stop and i == NK - 1),
                    )

        def mm_half_t(ps, m, j, movs, start, stop):
            """Same but moving operand is per-k tiles (hr)."""
            for k in range(NK):
                lhsT = w_ap(m, j, k)
                for b in range(NB):
                    nc.tensor.matmul(
                        ps[b],
                        lhsT,
                        movs[k][:, b * BW : (b + 1) * BW],
                        start=(start and k == 0),
                        stop=(stop and k == NK - 1),
                    )

        def activate(dst, ps, fn, bcol):
            for b in range(NB):
                nc.scalar.activation(
                    dst[:, b * BW : (b + 1) * BW], ps[b], fn,
                    bias=btile[:, bcol : bcol + 1],
                )

        for ch in range(N_CHUNKS):
            xc, hc = xts[ch], hts[ch]

            # ---- R phase, software-pipelined depth 2: the x-parts of
            # j+1/j+2 give the PE work while h^T (and U_r) are still in
            # flight at kernel start.  Live PSUM groups <= 3 (6 banks).
            hrs = []

            def finish_r(j, ps):
                mm_half(ps, 1, j, hc, start=False, stop=True)
                rtile = rp.tile([128, CW], BF16, name="rtile")
                activate(rtile, ps, AF.Sigmoid, j)
                hrtile = hrp.tile([128, CW], BF16, name="hrtile")
                nc.vector.tensor_mul(
                    hrtile, hc[:, j * CW : (j + 1) * CW], rtile
                )
                hrs.append(hrtile)

            ps_list = []
            for j in range(NH):
                ps = [pp.tile([128, BW], F32, name="ps") for _ in range(NB)]
                # first real matmuls consume x0's k-slices in DMA-arrival
                # order (rings A/B interleave) instead of stalling on k=0
                ko = [4, 0, 5, 1, 6, 2, 7, 3] if (ch == 0 and j == 0) else None
                mm_half(ps, 0, j, xc, start=True, stop=False, korder=ko)
                ps_list.append(ps)
                if j >= 2:
                    finish_r(j - 2, ps_list[j - 2])
            finish_r(NH - 2, ps_list[NH - 2])
            finish_r(NH - 1, ps_list[NH - 1])

            # ---- U + C + OUT fused per j ----
            for j in range(NH):
                psu = [pp.tile([128, BW], F32, name="ps") for _ in range(NB)]
                mm_half(psu, 2, j, xc, start=True, stop=False)
                mm_half(psu, 3, j, hc, start=False, stop=True)
                util = up.tile([128, CW], BF16, name="utile")
                activate(util, psu, AF.Sigmoid, NH + j)

                psc = [pp.tile([128, BW], F32, name="ps") for _ in range(NB)]
                mm_half(psc, 4, j, xc, start=True, stop=False)
                mm_half_t(psc, 5, j, hrs, start=False, stop=True)
                ctile = cp.tile([128, CW], BF16, name="ctile")
                activate(ctile, psc, AF.Tanh, 2 * NH + j)

                # per-bank combine + store: bank 0's chain overlaps bank 1's
                # activation, shrinking the end-of-kernel tail
                hj = hc[:, j * CW : (j + 1) * CW]
                t = op.tile([128, CW], F32, name="ttile")
                for b in range(NB):
                    sl = slice(b * BW, (b + 1) * BW)
                    nc.vector.tensor_sub(t[:, sl], ctile[:, sl], hj[:, sl])
                    nc.vector.tensor_mul(t[:, sl], util[:, sl], t[:, sl])
                    nc.vector.tensor_add(t[:, sl], t[:, sl], hj[:, sl])
                    nc.gpsimd.dma_start(
                        out[
                            j * 128 : (j + 1) * 128,
                            ch * CW + b * BW : ch * CW + (b + 1) * BW,
                        ],
                        t[:, sl],
                    )

    nc.compile()
    return nc


def pack_inputs(inputs):
    """Host-side shard + transpose + pack + cast. Per-core input maps."""
    x = np.asarray(inputs["x_t"], np.float32)
    h = np.asarray(inputs["h_prev"], np.float32)

    mats = [inputs["W_r"], inputs["U_r"], inputs["W_u"], inputs["U_u"],
            inputs["W"], inputs["U"]]
    wts = np.empty((6, 128, NH * NK * 128), ml_dtypes.bfloat16)
    for i, m in enumerate(mats):
        mt = np.asarray(m, np.float32).T.astype(ml_dtypes.bfloat16)  # [in, out]
        # [k,p,j,c] -> [p,j,k,c]
        wts[i] = (
            mt.reshape(NK, 128, NH, 128)
            .transpose(1, 2, 0, 3)
            .reshape(128, NH * NK * 128)
        )

    b_r = np.asarray(inputs["b_Wr"], np.float32) + np.asarray(inputs["b_Ur"], np.float32)
    b_u = np.asarray(inputs["b_Wu"], np.float32) + np.asarray(inputs["b_Uu"], np.float32)
    b_c = np.asarray(inputs["b_W"], np.float32) + np.asarray(inputs["b_U"], np.float32)
    bias = np.concatenate(
        [bb.reshape(NH, 128).T for bb in (b_r, b_u, b_c)], axis=1
    ).astype(np.float32)  # [128, 3*NH]

    def pack_bt(a_rows):  # [B_CORE, D] f32 -> [N_CHUNKS, 128, NK*CW] bf16
        at = np.ascontiguousarray(a_rows.T).astype(ml_dtypes.bfloat16)  # [D, B_CORE]
        o = np.empty((N_CHUNKS, 128, NK * CW), ml_dtypes.bfloat16)
        for ch in range(N_CHUNKS):
            o[ch] = (
                at[:, ch * CW : (ch + 1) * CW]
                .reshape(NK, 128, CW)
                .transpose(1, 0, 2)
                .reshape(128, NK * CW)
            )
        return o

    in_maps = []
    for c in range(N_CORES):
        rows = slice(c * B_CORE, (c + 1) * B_CORE)
        in_maps.append(
            {
                "xt": pack_bt(x[rows]),
                "ht": pack_bt(h[rows]),
                "wts": wts,
                "bias": bias,
            }
        )
    return in_maps


_NC_CACHE = {}


def _get_nc():
    if "nc" not in _NC_CACHE:
        _NC_CACHE["nc"] = build_nc()
    return _NC_CACHE["nc"]


def _run(inputs, **spmd_kwargs):
    nc = _get_nc()
    in_maps = pack_inputs(inputs)
    res = run_bass_kernel_spmd(nc, in_maps, list(range(N_CORES)), **spmd_kwargs)
    out = np.empty((B, D), np.float32)
    for c in range(N_CORES):
        out[c * B_CORE : (c + 1) * B_CORE, :] = res.results[c]["out"].T
    return out, [res]


def kernel(**inputs):
    out, _ = _run(inputs)
    return out


# revision 12
# speedup vs baseline: 1.3154x; 1.0932x over previous
"""GRU cell kernel for Trainium2, 8-core data-parallel, single dispatch.

Strategy
--------
Data-parallel on batch across 8 cores; each core handles 2048 rows in ONE
NEFF dispatch, processed as two in-kernel chunks of 1024 columns (SBUF
capacity).  All on-chip compute is in transposed space ([hidden, batch])
so matmul contractions land on SBUF partitions with no on-device
transposes:

    r^T = sigmoid(W_r @ x^T + U_r @ h^T + b_r)
    u^T = sigmoid(W_u @ x^T + U_u @ h^T + b_u)
    c^T = tanh   (W   @ x^T + U  @ (h.r)^T + b_c)
    o^T = h^T + u^T * (c^T - h^T)

Matmuls run in bf16.  v1 profiling showed the kernel was gated by load
DMAs: 409 per-tile descriptors (32-256KB each) serialized on the Sync
engine's single HWDGE ring at ~600ns/descriptor, leaving LDWEIGHTS
waiting 0.5-0.8us on weight arrival between every matmul pair (340ns
observed MM spacing vs the 226ns streaming floor).  Sub-64KB DMAs are
descriptor-dominated (~52GB/s); >=1MB DMAs hit 341-425GB/s.

So v2 host-prepacks everything into big [128, F] blocks and loads with a
handful of ~2MB DMAs split across BOTH HWDGE rings (weights on the
qAct ring via nc.scalar, x/h on the qSP ring via nc.sync) so weight
prefetch never queues behind input streaming.  All tiles are fully
SBUF-resident with no DMA-written slot ever recycled (this toolchain's
DMA descriptors carry exactly ONE sync wait, so a load needing a WAR
wait on top of its queue wait fails codegen).  Stores ride SWDGE
(gpsimd) queues with their single RAW wait.

The R phase is software-pipelined (x-parts of j+2 interleave with
h-parts of j) so the PE has x-side work while U_r/h^T are still in
flight at kernel start.  U/C/OUT are fused per j: u_j and c_j feed the
output combine immediately, so only r/u/c transients of 2 tiles each are
live and everything fits in ~196KB/partition of SBUF.
"""

import sys

sys.path.insert(0, "/opt/trn_rl_repo")

import numpy as np
import ml_dtypes
from contextlib import ExitStack

import concourse.bass as bass
import concourse.bacc as bacc
import concourse.mybir as mybir
from concourse import tile
from concourse.bass_utils import run_bass_kernel_spmd

BF16 = mybir.dt.bfloat16
F32 = mybir.dt.float32
AF = mybir.ActivationFunctionType

N_CORES = 8
B = 16384
D = 1024  # IN == H
B_CORE = B // N_CORES  # 2048 rows per core
N_CHUNKS = 2
CW = B_CORE // N_CHUNKS  # 1024 batch columns per chunk
BW = 512  # matmul moving width (one fp32 PSUM bank)
NH = D // 128  # 8 hidden row-tiles
NK = D // 128  # 8 contraction tiles
NB = CW // BW  # 2 PSUM banks per chunk row


def build_nc():
    """Build the SPMD per-core Bass program.

    DRAM inputs (all host-prepacked, partition-major):
      wts  [6, 128, NH*NK*128] bf16 : per mat m, wts[m][p, (j*NK+k)*128+c]
                                      = M_m.T[k*128+p, j*128+c]
                                      (mats: 0=W_r 1=U_r 2=W_u 3=U_u 4=W 5=U)
      xt   [N_CHUNKS, 128, NK*CW]   : xt[ch][p, k*CW+c] = x.T[k*128+p, ch*CW+c]
      ht   [N_CHUNKS, 128, NK*CW]   : same for h_prev
      bias [128, 3*NH] f32          : bias[p, g*NH+j] = b_g[j*128+p]
    Output:
      out  [D, B_CORE] f32          : out[d, b] = o.T[d, b]
    """
    nc = bacc.Bacc("TRN2", target_bir_lowering=False)
    wts = nc.dram_tensor("wts", [6, 128, NH * NK * 128], BF16, kind="ExternalInput")
    xt = nc.dram_tensor("xt", [N_CHUNKS, 128, NK * CW], BF16, kind="ExternalInput")
    ht = nc.dram_tensor("ht", [N_CHUNKS, 128, NK * CW], BF16, kind="ExternalInput")
    bias = nc.dram_tensor("bias", [128, 3 * NH], F32, kind="ExternalInput")
    out = nc.dram_tensor("out", [D, B_CORE], F32, kind="ExternalOutput")

    with tile.TileContext(nc) as tc, ExitStack() as ctx:
        wp = ctx.enter_context(tc.tile_pool(name="wp", bufs=6))
        xp = ctx.enter_context(tc.tile_pool(name="xp", bufs=N_CHUNKS))
        hp = ctx.enter_context(tc.tile_pool(name="hp", bufs=N_CHUNKS))
        hrp = ctx.enter_context(tc.tile_pool(name="hrp", bufs=NH))
        rp = ctx.enter_context(tc.tile_pool(name="rp", bufs=2))
        up = ctx.enter_context(tc.tile_pool(name="up", bufs=2))
        cp = ctx.enter_context(tc.tile_pool(name="cp", bufs=2))
        op = ctx.enter_context(tc.tile_pool(name="op", bufs=2))
        bp = ctx.enter_context(tc.tile_pool(name="bp", bufs=1))
        pp = ctx.enter_context(tc.tile_pool(name="pp", bufs=8, space="PSUM"))

        # NOTE: no PE warm-up matmuls here.  A warm-up group into one PSUM
        # slot shifts the pool ring so every real accumulation group's bank
        # pair lands misaligned (odd,even) — measured +43ns on EVERY matmul
        # (259ns vs the 216ns streaming floor).  Same-bank warm-up MMs also
        # serialize at ~512ns and delay the first real matmul past the
        # point where its DMAs have landed.

        # ---- loads: split across the two HWDGE rings in demand order.
        # First-chunk tensors are j/k-sliced (~256KB pieces) so the first
        # matmul's dependency set is ~0.5MB, not 4MB.
        wtiles = [
            wp.tile([128, NH * NK * 128], BF16, name="wtile") for _ in range(6)
        ]
        xts, hts = [], []
        for ch in range(N_CHUNKS):
            xts.append(xp.tile([128, NK * CW], BF16, name="xtile"))
            hts.append(hp.tile([128, NK * CW], BF16, name="htile"))
        btile = bp.tile([128, 3 * NH], F32, name="btile")

        JW = NK * 128  # 1024 cols per j-block of a weight tile

        def ld_w(eng, m, j):
            eng.dma_start(
                wtiles[m][:, j * JW : (j + 1) * JW], wts[m, :, j * JW : (j + 1) * JW]
            )

        def ld_bt(eng, tile_, dram, ch, k):
            eng.dma_start(
                tile_[:, k * CW : (k + 1) * CW], dram[ch, :, k * CW : (k + 1) * CW]
            )

        A, Bq = nc.scalar, nc.sync
        # ring A (qAct): W_r j-sliced + U_r j-sliced interleaved, then the
        # three late-phase mats whole; x0's first ks ride here too so the
        # j0 x-part can start ~10us in.
        ld_w(A, 0, 0)
        for k in range(4):
            ld_bt(A, xts[0], xt, 0, k)
        for j in (1, 2, 3, 4):
            ld_w(A, 0, j)
        ld_w(A, 1, 0)
        ld_w(A, 0, 5)
        ld_w(A, 1, 1)
        ld_w(A, 0, 6)
        ld_w(A, 1, 2)
        ld_w(A, 0, 7)
        for j in (3, 4, 5, 6, 7):
            ld_w(A, 1, j)
        for m in (2, 3, 4, 5):
            A.dma_start(wtiles[m], wts[m, :, :])
        # ring B (qSP): bias, rest of x0, h0 k-sliced, chunk-1 whole.
        Bq.dma_start(btile, bias[:, :])
        for k in range(4, NK):
            ld_bt(Bq, xts[0], xt, 0, k)
        for k in range(NK):
            ld_bt(Bq, hts[0], ht, 0, k)
        Bq.dma_start(xts[1], xt[1, :, :])
        Bq.dma_start(hts[1], ht[1, :, :])

        def w_ap(m, j, k):
            return wtiles[m][:, (j * NK + k) * 128 : (j * NK + k + 1) * 128]

        def mm_half(ps, m, j, mov, start, stop, korder=None):
            """One mat's contraction into NB psum banks (bank innermost)."""
            for i, k in enumerate(korder or range(NK)):
                lhsT = w_ap(m, j, k)
                for b in range(NB):
                    nc.tensor.matmul(
                        ps[b],
                        lhsT,
                        mov[:, k * CW + b * BW : k * CW + (b + 1) * BW],
                        start=(start and i == 0),
                        stop=(stop and i == NK - 1),
                    )

        def mm_half_t(ps, m, j, movs, start, stop):
            """Same but moving operand is per-k tiles (hr)."""
            for k in range(NK):
                lhsT = w_ap(m, j, k)
                for b in range(NB):
                    nc.tensor.matmul(
                        ps[b],
                        lhsT,
                        movs[k][:, b * BW : (b + 1) * BW],
                        start=(start and k == 0),
                        stop=(stop and k == NK - 1),
                    )

        def activate(dst, ps, fn, bcol):
            for b in range(NB):
                nc.scalar.activation(
                    dst[:, b * BW : (b + 1) * BW], ps[b], fn,
                    bias=btile[:, bcol : bcol + 1],
                )

        for ch in range(N_CHUNKS):
            xc, hc = xts[ch], hts[ch]

            # ---- R phase, software-pipelined depth 2: the x-parts of
            # j+1/j+2 give the PE work while h^T (and U_r) are still in
            # flight at kernel start.  Live PSUM groups <= 3 (6 banks).
            hrs = []

            def finish_r(j, ps):
                mm_half(ps, 1, j, hc, start=False, stop=True)
                rtile = rp.tile([128, CW], BF16, name="rtile")
                activate(rtile, ps, AF.Sigmoid, j)
                hrtile = hrp.tile([128, CW], BF16, name="hrtile")
                nc.vector.tensor_mul(
                    hrtile, hc[:, j * CW : (j + 1) * CW], rtile
                )
                hrs.append(hrtile)

            ps_list = []
            for j in range(NH):
                ps = [pp.tile([128, BW], F32, name="ps") for _ in range(NB)]
                # first real matmuls consume x0's k-slices in DMA-arrival
                # order (rings A/B interleave) instead of stalling on k=0
                ko = [4, 0, 5, 1, 6, 2, 7, 3] if (ch == 0 and j == 0) else None
                mm_half(ps, 0, j, xc, start=True, stop=False, korder=ko)
                ps_list.append(ps)
                if j >= 2:
                    finish_r(j - 2, ps_list[j - 2])
            finish_r(NH - 2, ps_list[NH - 2])
            finish_r(NH - 1, ps_list[NH - 1])

            # ---- U + C + OUT fused per j ----
            for j in range(NH):
                psu = [pp.tile([128, BW], F32, name="ps") for _ in range(NB)]
                mm_half(psu, 2, j, xc, start=True, stop=False)
                mm_half(psu, 3, j, hc, start=False, stop=True)
                util = up.tile([128, CW], BF16, name="utile")
                activate(util, psu, AF.Sigmoid, NH + j)

                psc = [pp.tile([128, BW], F32, name="ps") for _ in range(NB)]
                mm_half(psc, 4, j, xc, start=True, stop=False)
                mm_half_t(psc, 5, j, hrs, start=False, stop=True)
                ctile = cp.tile([128, CW], BF16, name="ctile")
                activate(ctile, psc, AF.Tanh, 2 * NH + j)

                # per-bank combine + store: bank 0's chain overlaps bank 1's
                # activation, shrinking the end-of-kernel tail
                hj = hc[:, j * CW : (j + 1) * CW]
                t = op.tile([128, CW], F32, name="ttile")
                for b in range(NB):
                    sl = slice(b * BW, (b + 1) * BW)
                    nc.vector.tensor_sub(t[:, sl], ctile[:, sl], hj[:, sl])
                    nc.vector.tensor_mul(t[:, sl], util[:, sl], t[:, sl])
                    nc.vector.tensor_add(t[:, sl], t[:, sl], hj[:, sl])
                    nc.sync.dma_start(
                        out[
                            j * 128 : (j + 1) * 128,
                            ch * CW + b * BW : ch * CW + (b + 1) * BW,
                        ],
                        t[:, sl],
                    )

    nc.compile()
    return nc


def pack_inputs(inputs):
    """Host-side shard + transpose + pack + cast. Per-core input maps."""
    x = np.asarray(inputs["x_t"], np.float32)
    h = np.asarray(inputs["h_prev"], np.float32)

    mats = [inputs["W_r"], inputs["U_r"], inputs["W_u"], inputs["U_u"],
            inputs["W"], inputs["U"]]
    wts = np.empty((6, 128, NH * NK * 128), ml_dtypes.bfloat16)
    for i, m in enumerate(mats):
        mt = np.asarray(m, np.float32).T.astype(ml_dtypes.bfloat16)  # [in, out]
        # [k,p,j,c] -> [p,j,k,c]
        wts[i] = (
            mt.reshape(NK, 128, NH, 128)
            .transpose(1, 2, 0, 3)
            .reshape(128, NH * NK * 128)
        )

    b_r = np.asarray(inputs["b_Wr"], np.float32) + np.asarray(inputs["b_Ur"], np.float32)
    b_u = np.asarray(inputs["b_Wu"], np.float32) + np.asarray(inputs["b_Uu"], np.float32)
    b_c = np.asarray(inputs["b_W"], np.float32) + np.asarray(inputs["b_U"], np.float32)
    bias = np.concatenate(
        [bb.reshape(NH, 128).T for bb in (b_r, b_u, b_c)], axis=1
    ).astype(np.float32)  # [128, 3*NH]

    def pack_bt(a_rows):  # [B_CORE, D] f32 -> [N_CHUNKS, 128, NK*CW] bf16
        at = np.ascontiguousarray(a_rows.T).astype(ml_dtypes.bfloat16)  # [D, B_CORE]
        o = np.empty((N_CHUNKS, 128, NK * CW), ml_dtypes.bfloat16)
        for ch in range(N_CHUNKS):
            o[ch] = (
                at[:, ch * CW : (ch + 1) * CW]
                .reshape(NK, 128, CW)
                .transpose(1, 0, 2)
                .reshape(128, NK * CW)
            )
        return o

    in_maps = []
    for c in range(N_CORES):
        rows = slice(c * B_CORE, (c + 1) * B_CORE)
        in_maps.append(
            {
                "xt": pack_bt(x[rows]),
                "ht": pack_bt(h[rows]),
                "wts": wts,
                "bias": bias,
            }
        )
    return in_maps


_NC_CACHE = {}


def _get_nc():
    if "nc" not in _NC_CACHE:
        _NC_CACHE["nc"] = build_nc()
    return _NC_CACHE["nc"]


def _run(inputs, **spmd_kwargs):
    nc = _get_nc()
    in_maps = pack_inputs(inputs)
    res = run_bass_kernel_spmd(nc, in_maps, list(range(N_CORES)), **spmd_kwargs)
    out = np.empty((B, D), np.float32)
    for c in range(N_CORES):
        out[c * B_CORE : (c + 1) * B_CORE, :] = res.results[c]["out"].T
    return out, [res]


def kernel(**inputs):
    out, _ = _run(inputs)
    return out


# revision 13
# speedup vs baseline: 1.3651x; 1.0378x over previous
"""GRU cell kernel for Trainium2 — v4: bf16 + partial-fp8 (DoubleRow).

Same structure as kernel.py (single dispatch, 8-core DP, 2 chunks,
transposed space, big demand-ordered DMAs), but a leading slice of each
matmul contraction runs in fp8e4m3 with DoubleRow perf mode (2 fp8
weights per PE cell -> 256-deep contraction per instruction, ~1.8x the
bf16 rate for that slice):

    W_r/U_r: first 512 of 1024 contraction rows in fp8  (p=2 pairs)
    W_u/U_u/W/U: first 256 rows in fp8                  (p=1 pair)

The sigmoid/tanh nonlinearities damp the extra quantization noise;
simulated end-to-end rel err 1.54e-2 vs the 2e-2 gate (pure bf16 is
3.7e-3).  bf16 weight tiles drop their fp8-covered k-slices so
everything still fits in ~196KB/partition of SBUF.
"""

import sys

sys.path.insert(0, "/opt/trn_rl_repo")

import numpy as np
import ml_dtypes
from contextlib import ExitStack

import concourse.bass as bass
import concourse.bacc as bacc
import concourse.mybir as mybir
from concourse import tile
from concourse.bass_utils import run_bass_kernel_spmd

BF16 = mybir.dt.bfloat16
FP8 = mybir.dt.float8e4
F32 = mybir.dt.float32
AF = mybir.ActivationFunctionType
DR = mybir.MatmulPerfMode.DoubleRow
NPF8 = ml_dtypes.float8_e4m3

N_CORES = 8
B = 16384
D = 1024
B_CORE = B // N_CORES
N_CHUNKS = 2
CW = B_CORE // N_CHUNKS  # 1024
BW = 512
NH = D // 128
NK = D // 128
NB = CW // BW
# fp8 pairs per mat: 0=W_r 1=U_r 2=W_u 3=U_u 4=W 5=U
P8 = (2, 2, 1, 1, 1, 1)
KS = tuple(2 * p for p in P8)  # first bf16 k-tile per mat


def build_nc():
    nc = bacc.Bacc("TRN2", target_bir_lowering=False)
    # slim bf16 weights: mat m keeps k-tiles [KS[m], NK)
    wtsa = nc.dram_tensor("wtsa", [2, 128, NH * (NK - KS[0]) * 128], BF16,
                          kind="ExternalInput")
    wtsb = nc.dram_tensor("wtsb", [4, 128, NH * (NK - KS[2]) * 128], BF16,
                          kind="ExternalInput")
    w8a = nc.dram_tensor("w8a", [2, 128, 2 * P8[0], NH * 128], FP8,
                         kind="ExternalInput")
    w8b = nc.dram_tensor("w8b", [4, 128, 2 * P8[2], NH * 128], FP8,
                         kind="ExternalInput")
    xt = nc.dram_tensor("xt", [N_CHUNKS, 128, NK * CW], BF16, kind="ExternalInput")
    ht = nc.dram_tensor("ht", [N_CHUNKS, 128, NK * CW], BF16, kind="ExternalInput")
    x8 = nc.dram_tensor("x8", [N_CHUNKS, 128, 4, CW], FP8, kind="ExternalInput")
    h8 = nc.dram_tensor("h8", [N_CHUNKS, 128, 4, CW], FP8, kind="ExternalInput")
    bias = nc.dram_tensor("bias", [128, 3 * NH], F32, kind="ExternalInput")
    out = nc.dram_tensor("out", [D, B_CORE], F32, kind="ExternalOutput")

    with tile.TileContext(nc) as tc, ExitStack() as ctx:
        wp = ctx.enter_context(tc.tile_pool(name="wp", bufs=1))
        w8p = ctx.enter_context(tc.tile_pool(name="w8p", bufs=1))
        xp = ctx.enter_context(tc.tile_pool(name="xp", bufs=N_CHUNKS))
        hp = ctx.enter_context(tc.tile_pool(name="hp", bufs=N_CHUNKS))
        x8p = ctx.enter_context(tc.tile_pool(name="x8p", bufs=N_CHUNKS))
        h8p = ctx.enter_context(tc.tile_pool(name="h8p", bufs=N_CHUNKS))
        hrp = ctx.enter_context(tc.tile_pool(name="hrp", bufs=NH - 2))
        hr8p = ctx.enter_context(tc.tile_pool(name="hr8p", bufs=2))
        rp = ctx.enter_context(tc.tile_pool(name="rp", bufs=2))
        up = ctx.enter_context(tc.tile_pool(name="up", bufs=2))
        cp = ctx.enter_context(tc.tile_pool(name="cp", bufs=2))
        op = ctx.enter_context(tc.tile_pool(name="op", bufs=2))
        bp = ctx.enter_context(tc.tile_pool(name="bp", bufs=1))
        pp = ctx.enter_context(tc.tile_pool(name="pp", bufs=8, space="PSUM"))

        # tiles (distinct tag per mat via bufs=1 + unique names is fine here:
        # each is a singleton)
        wtiles = []
        for m in range(6):
            jw = (NK - KS[m]) * 128
            wtiles.append(wp.tile([128, NH * jw], BF16, name=f"w{m}", bufs=1))
        w8tiles = []
        for m in range(6):
            w8tiles.append(
                w8p.tile([128, 2 * P8[m], NH * 128], FP8, name=f"w8_{m}", bufs=1)
            )
        xts, hts, x8ts, h8ts = [], [], [], []
        for ch in range(N_CHUNKS):
            xts.append(xp.tile([128, NK * CW], BF16, name="xtile"))
            hts.append(hp.tile([128, NK * CW], BF16, name="htile"))
            x8ts.append(x8p.tile([128, 4, CW], FP8, name="x8tile"))
            h8ts.append(h8p.tile([128, 4, CW], FP8, name="h8tile"))
        btile = bp.tile([128, 3 * NH], F32, name="btile")

        def ld_w(eng, m, j):
            jw = (NK - KS[m]) * 128
            dram = wtsa if m < 2 else wtsb
            eng.dma_start(
                wtiles[m][:, j * jw : (j + 1) * jw],
                dram[m if m < 2 else m - 2, :, j * jw : (j + 1) * jw],
            )

        def ld_bt(eng, tile_, dram, ch, k):
            eng.dma_start(
                tile_[:, k * CW : (k + 1) * CW], dram[ch, :, k * CW : (k + 1) * CW]
            )

        A, Bq = nc.scalar, nc.sync
        # Ring plans follow the R-phase consumption order (fp8 compute is
        # ~25% faster per j than v3, so the v3 schedule starved the PE into
        # HAM cold restarts — two 7-8.5us gaps at 36/57us).  Ring A carries
        # x0's bf16 ks interleaved with the W_r/U_r j-slices; ring B carries
        # the fp8 input blocks + h0; the late-phase bf16 W/U split across
        # both rings so neither is the straggler at UC start.
        A.dma_start(w8tiles[0], w8a[0])
        ld_bt(A, xts[0], xt, 0, 4)
        ld_bt(A, xts[0], xt, 0, 5)
        ld_w(A, 0, 0)
        ld_bt(A, xts[0], xt, 0, 6)
        ld_bt(A, xts[0], xt, 0, 7)
        A.dma_start(w8tiles[1], w8a[1])
        ld_w(A, 0, 1)
        ld_w(A, 1, 0)
        ld_bt(A, xts[0], xt, 0, 2)
        ld_bt(A, xts[0], xt, 0, 3)
        ld_w(A, 0, 2)
        ld_w(A, 1, 1)
        ld_w(A, 0, 3)
        ld_w(A, 1, 2)
        ld_w(A, 0, 4)
        ld_w(A, 1, 3)
        ld_w(A, 0, 5)
        ld_w(A, 1, 4)
        ld_w(A, 0, 6)
        ld_w(A, 1, 5)
        ld_w(A, 0, 7)
        ld_w(A, 1, 6)
        ld_w(A, 1, 7)
        for m in (2, 3, 4, 5):
            A.dma_start(w8tiles[m], w8b[m - 2])
        A.dma_start(wtiles[2], wtsb[0])
        A.dma_start(wtiles[3], wtsb[1])
        # ring B: bias, chunk-0 fp8 inputs, h0 (matmul ks first), the two
        # late bf16 mats, then all of chunk 1.
        Bq.dma_start(btile, bias[:, :])
        Bq.dma_start(x8ts[0], x8[0])
        Bq.dma_start(h8ts[0], h8[0])
        for k in (4, 5, 6, 7, 0, 1, 2, 3):
            ld_bt(Bq, hts[0], ht, 0, k)
        Bq.dma_start(wtiles[4], wtsb[2])
        Bq.dma_start(wtiles[5], wtsb[3])
        Bq.dma_start(x8ts[1], x8[1])
        Bq.dma_start(h8ts[1], h8[1])
        for k in (2, 3, 4, 5, 6, 7):
            ld_bt(Bq, xts[1], xt, 1, k)
        for k in range(NK):
            ld_bt(Bq, hts[1], ht, 1, k)

        def w_ap(m, j, k):
            jw = NK - KS[m]
            return wtiles[m][:, (j * jw + (k - KS[m])) * 128 : (j * jw + (k - KS[m]) + 1) * 128]

        def mm_half(ps, m, j, mov, mov8, start, stop):
            """fp8 DoubleRow pairs then bf16 k-tiles into NB psum banks.

            mov: bf16 [128, NK*CW] tile; mov8: fp8 [128, >=2*P8[m], CW]."""
            for q in range(P8[m]):
                lhsT = w8tiles[m][:, 2 * q : 2 * q + 2, j * 128 : (j + 1) * 128]
                for b in range(NB):
                    nc.tensor.matmul(
                        ps[b],
                        lhsT,
                        mov8[:, 2 * q : 2 * q + 2, b * BW : (b + 1) * BW],
                        start=(start and q == 0),
                        stop=False,
                        perf_mode=DR,
                    )
            for k in range(KS[m], NK):
                lhsT = w_ap(m, j, k)
                for b in range(NB):
                    nc.tensor.matmul(
                        ps[b],
                        lhsT,
                        mov[:, k * CW + b * BW : k * CW + (b + 1) * BW],
                        start=False,
                        stop=(stop and k == NK - 1),
                    )

        def activate(dst, ps, fn, bcol):
            for b in range(NB):
                nc.scalar.activation(
                    dst[:, b * BW : (b + 1) * BW], ps[b], fn,
                    bias=btile[:, bcol : bcol + 1],
                )

        for ch in range(N_CHUNKS):
            xc, hc = xts[ch], hts[ch]
            x8c, h8c = x8ts[ch], h8ts[ch]

            hrs = [None] * NH
            hr8 = hr8p.tile([128, 2, CW], FP8, name="hr8tile")

            def finish_r(j, ps):
                mm_half(ps, 1, j, hc, h8c, start=False, stop=True)
                rtile = rp.tile([128, CW], BF16, name="rtile")
                activate(rtile, ps, AF.Sigmoid, j)
                hj = hc[:, j * CW : (j + 1) * CW]
                if j < 2:
                    # U's fp8 slice: write h*r straight to fp8 (pair 0)
                    nc.vector.tensor_mul(hr8[:, j : j + 1, :], hj, rtile)
                else:
                    hrtile = hrp.tile([128, CW], BF16, name="hrtile")
                    nc.vector.tensor_mul(hrtile, hj, rtile)
                    hrs[j] = hrtile

            ps_list = []
            for j in range(NH):
                ps = [pp.tile([128, BW], F32, name="ps") for _ in range(NB)]
                mm_half(ps, 0, j, xc, x8c, start=True, stop=False)
                ps_list.append(ps)
                if j >= 2:
                    finish_r(j - 2, ps_list[j - 2])
            finish_r(NH - 2, ps_list[NH - 2])
            finish_r(NH - 1, ps_list[NH - 1])

            for j in range(NH):
                psu = [pp.tile([128, BW], F32, name="ps") for _ in range(NB)]
                mm_half(psu, 2, j, xc, x8c, start=True, stop=False)
                mm_half(psu, 3, j, hc, h8c, start=False, stop=True)
                util = up.tile([128, CW], BF16, name="utile")
                activate(util, psu, AF.Sigmoid, NH + j)

                psc = [pp.tile([128, BW], F32, name="ps") for _ in range(NB)]
                mm_half(psc, 4, j, xc, x8c, start=True, stop=False)
                # U-side of c: fp8 pair 0 from hr8, bf16 k>=2 from hrs
                lhsT8 = w8tiles[5][:, 0:2, j * 128 : (j + 1) * 128]
                for b in range(NB):
                    nc.tensor.matmul(
                        psc[b], lhsT8, hr8[:, :, b * BW : (b + 1) * BW],
                        start=False, stop=False, perf_mode=DR,
                    )
                for k in range(2, NK):
                    lhsT = w_ap(5, j, k)
                    for b in range(NB):
                        nc.tensor.matmul(
                            psc[b], lhsT, hrs[k][:, b * BW : (b + 1) * BW],
                            start=False, stop=(k == NK - 1),
                        )
                ctile = cp.tile([128, CW], BF16, name="ctile")
                activate(ctile, psc, AF.Tanh, 2 * NH + j)

                hj = hc[:, j * CW : (j + 1) * CW]
                t = op.tile([128, CW], F32, name="ttile")
                for b in range(NB):
                    sl = slice(b * BW, (b + 1) * BW)
                    nc.vector.tensor_sub(t[:, sl], ctile[:, sl], hj[:, sl])
                    nc.vector.tensor_mul(t[:, sl], util[:, sl], t[:, sl])
                    nc.vector.tensor_add(t[:, sl], t[:, sl], hj[:, sl])
                    nc.sync.dma_start(
                        out[
                            j * 128 : (j + 1) * 128,
                            ch * CW + b * BW : ch * CW + (b + 1) * BW,
                        ],
                        t[:, sl],
                    )

    nc.compile()
    return nc


def pack_inputs(inputs):
    x = np.asarray(inputs["x_t"], np.float32)
    h = np.asarray(inputs["h_prev"], np.float32)

    mats = [inputs["W_r"], inputs["U_r"], inputs["W_u"], inputs["U_u"],
            inputs["W"], inputs["U"]]
    wtsa = np.empty((2, 128, NH * (NK - KS[0]) * 128), ml_dtypes.bfloat16)
    wtsb = np.empty((4, 128, NH * (NK - KS[2]) * 128), ml_dtypes.bfloat16)
    w8a = np.empty((2, 128, 2 * P8[0], NH * 128), NPF8)
    w8b = np.empty((4, 128, 2 * P8[2], NH * 128), NPF8)
    for i, m in enumerate(mats):
        mt = np.asarray(m, np.float32).T  # [in(k), out(j)]
        t4 = mt.reshape(NK, 128, NH, 128)  # [k, p, j, c]
        ks = KS[i]
        bf = (
            t4[ks:]
            .transpose(1, 2, 0, 3)  # [p, j, k', c]
            .reshape(128, NH * (NK - ks) * 128)
            .astype(ml_dtypes.bfloat16)
        )
        f8 = (
            t4[:ks]
            .transpose(1, 0, 2, 3)  # [p, kk, j, c]
            .reshape(128, ks, NH * 128)
            .astype(NPF8)
        )
        if i < 2:
            wtsa[i] = bf
            w8a[i] = f8
        else:
            wtsb[i - 2] = bf
            w8b[i - 2] = f8

    b_r = np.asarray(inputs["b_Wr"], np.float32) + np.asarray(inputs["b_Ur"], np.float32)
    b_u = np.asarray(inputs["b_Wu"], np.float32) + np.asarray(inputs["b_Uu"], np.float32)
    b_c = np.asarray(inputs["b_W"], np.float32) + np.asarray(inputs["b_U"], np.float32)
    bias = np.concatenate(
        [bb.reshape(NH, 128).T for bb in (b_r, b_u, b_c)], axis=1
    ).astype(np.float32)

    def pack_bt(a_rows):
        at = np.ascontiguousarray(a_rows.T)  # [D, B_CORE] f32
        o = np.empty((N_CHUNKS, 128, NK * CW), ml_dtypes.bfloat16)
        o8 = np.empty((N_CHUNKS, 128, 4, CW), NPF8)
        for ch in range(N_CHUNKS):
            blk = at[:, ch * CW : (ch + 1) * CW].reshape(NK, 128, CW)
            o[ch] = blk.transpose(1, 0, 2).reshape(128, NK * CW).astype(ml_dtypes.bfloat16)
            o8[ch] = blk[:4].transpose(1, 0, 2).astype(NPF8)
        return o, o8

    in_maps = []
    for c in range(N_CORES):
        rows = slice(c * B_CORE, (c + 1) * B_CORE)
        xb, xf8 = pack_bt(x[rows])
        hb, hf8 = pack_bt(h[rows])
        in_maps.append(
            {
                "xt": xb, "ht": hb, "x8": xf8, "h8": hf8,
                "wtsa": wtsa, "wtsb": wtsb, "w8a": w8a, "w8b": w8b,
                "bias": bias,
            }
        )
    return in_maps


_NC_CACHE = {}


def _get_nc():
    if "nc" not in _NC_CACHE:
        _NC_CACHE["nc"] = build_nc()
    return _NC_CACHE["nc"]


def _run(inputs, **spmd_kwargs):
    nc = _get_nc()
    in_maps = pack_inputs(inputs)
    res = run_bass_kernel_spmd(nc, in_maps, list(range(N_CORES)), **spmd_kwargs)
    out = np.empty((B, D), np.float32)
    for c in range(N_CORES):
        out[c * B_CORE : (c + 1) * B_CORE, :] = res.results[c]["out"].T
    return out, [res]


def kernel(**inputs):
    out, _ = _run(inputs)
    return out


# revision 16
# speedup vs baseline: 1.4619x; 1.0709x over previous
"""GRU cell kernel for Trainium2 — v4: bf16 + partial-fp8 (DoubleRow).

Same structure as kernel.py (single dispatch, 8-core DP, 2 chunks,
transposed space, big demand-ordered DMAs), but a leading slice of each
matmul contraction runs in fp8e4m3 with DoubleRow perf mode (2 fp8
weights per PE cell -> 256-deep contraction per instruction, ~1.8x the
bf16 rate for that slice):

    W_r/U_r: first 512 of 1024 contraction rows in fp8  (p=2 pairs)
    W_u/U_u/W/U: first 256 rows in fp8                  (p=1 pair)

The sigmoid/tanh nonlinearities damp the extra quantization noise;
simulated end-to-end rel err 1.54e-2 vs the 2e-2 gate (pure bf16 is
3.7e-3).  bf16 weight tiles drop their fp8-covered k-slices so
everything still fits in ~196KB/partition of SBUF.
"""

import sys

sys.path.insert(0, "/opt/trn_rl_repo")

import numpy as np
import ml_dtypes
from contextlib import ExitStack

import concourse.bass as bass
import concourse.bacc as bacc
import concourse.mybir as mybir
from concourse import tile
from concourse.bass_utils import run_bass_kernel_spmd

BF16 = mybir.dt.bfloat16
FP8 = mybir.dt.float8e4
F32 = mybir.dt.float32
AF = mybir.ActivationFunctionType
DR = mybir.MatmulPerfMode.DoubleRow
NPF8 = ml_dtypes.float8_e4m3

N_CORES = 8
B = 16384
D = 1024
B_CORE = B // N_CORES
N_CHUNKS = 2
CW = B_CORE // N_CHUNKS  # 1024
BW = 512
NH = D // 128
NK = D // 128
NB = CW // BW
# fp8 pairs per mat: 0=W_r 1=U_r 2=W_u 3=U_u 4=W 5=U
P8 = (2, 2, 1, 1, 1, 1)
KS = tuple(2 * p for p in P8)  # first bf16 k-tile per mat


def build_nc():
    nc = bacc.Bacc("TRN2", target_bir_lowering=False)
    # slim bf16 weights: mat m keeps k-tiles [KS[m], NK)
    wtsa = nc.dram_tensor("wtsa", [2, 128, NH * (NK - KS[0]) * 128], BF16,
                          kind="ExternalInput")
    wtsb = nc.dram_tensor("wtsb", [4, 128, NH * (NK - KS[2]) * 128], BF16,
                          kind="ExternalInput")
    w8a = nc.dram_tensor("w8a", [2, 128, 2 * P8[0], NH * 128], FP8,
                         kind="ExternalInput")
    w8b = nc.dram_tensor("w8b", [4, 128, 2 * P8[2], NH * 128], FP8,
                         kind="ExternalInput")
    xt = nc.dram_tensor("xt", [N_CHUNKS, 128, NK * CW], BF16, kind="ExternalInput")
    ht = nc.dram_tensor("ht", [N_CHUNKS, 128, NK * CW], BF16, kind="ExternalInput")
    x8 = nc.dram_tensor("x8", [N_CHUNKS, 128, 4, CW], FP8, kind="ExternalInput")
    h8 = nc.dram_tensor("h8", [N_CHUNKS, 128, 4, CW], FP8, kind="ExternalInput")
    bias = nc.dram_tensor("bias", [128, 3 * NH], F32, kind="ExternalInput")
    out = nc.dram_tensor("out", [D, B_CORE], F32, kind="ExternalOutput")

    with tile.TileContext(nc) as tc, ExitStack() as ctx:
        wp = ctx.enter_context(tc.tile_pool(name="wp", bufs=1))
        w8p = ctx.enter_context(tc.tile_pool(name="w8p", bufs=1))
        xp = ctx.enter_context(tc.tile_pool(name="xp", bufs=N_CHUNKS))
        hp = ctx.enter_context(tc.tile_pool(name="hp", bufs=N_CHUNKS))
        x8p = ctx.enter_context(tc.tile_pool(name="x8p", bufs=N_CHUNKS))
        h8p = ctx.enter_context(tc.tile_pool(name="h8p", bufs=N_CHUNKS))
        hrp = ctx.enter_context(tc.tile_pool(name="hrp", bufs=NH - 2))
        hr8p = ctx.enter_context(tc.tile_pool(name="hr8p", bufs=2))
        rp = ctx.enter_context(tc.tile_pool(name="rp", bufs=2))
        up = ctx.enter_context(tc.tile_pool(name="up", bufs=2))
        cp = ctx.enter_context(tc.tile_pool(name="cp", bufs=2))
        op = ctx.enter_context(tc.tile_pool(name="op", bufs=2))
        bp = ctx.enter_context(tc.tile_pool(name="bp", bufs=1))
        pp = ctx.enter_context(tc.tile_pool(name="pp", bufs=8, space="PSUM"))

        # tiles (distinct tag per mat via bufs=1 + unique names is fine here:
        # each is a singleton)
        wtiles = []
        for m in range(6):
            jw = (NK - KS[m]) * 128
            wtiles.append(wp.tile([128, NH * jw], BF16, name=f"w{m}", bufs=1))
        w8tiles = []
        for m in range(6):
            w8tiles.append(
                w8p.tile([128, 2 * P8[m], NH * 128], FP8, name=f"w8_{m}", bufs=1)
            )
        xts, hts, x8ts, h8ts = [], [], [], []
        for ch in range(N_CHUNKS):
            xts.append(xp.tile([128, NK * CW], BF16, name="xtile"))
            hts.append(hp.tile([128, NK * CW], BF16, name="htile"))
            x8ts.append(x8p.tile([128, 4, CW], FP8, name="x8tile"))
            h8ts.append(h8p.tile([128, 4, CW], FP8, name="h8tile"))
        btile = bp.tile([128, 3 * NH], F32, name="btile")

        def ld_w(eng, m, j):
            jw = (NK - KS[m]) * 128
            dram = wtsa if m < 2 else wtsb
            eng.dma_start(
                wtiles[m][:, j * jw : (j + 1) * jw],
                dram[m if m < 2 else m - 2, :, j * jw : (j + 1) * jw],
            )

        def ld_bt(eng, tile_, dram, ch, k):
            eng.dma_start(
                tile_[:, k * CW : (k + 1) * CW], dram[ch, :, k * CW : (k + 1) * CW]
            )

        A, Bq = nc.scalar, nc.sync
        # Ring plans follow the R-phase consumption order (fp8 compute is
        # ~25% faster per j than v3, so the v3 schedule starved the PE into
        # HAM cold restarts — two 7-8.5us gaps at 36/57us).  Ring A carries
        # x0's bf16 ks interleaved with the W_r/U_r j-slices; ring B carries
        # the fp8 input blocks + h0; the late-phase bf16 W/U split across
        # both rings so neither is the straggler at UC start.
        A.dma_start(w8tiles[0], w8a[0])
        ld_bt(A, xts[0], xt, 0, 4)
        ld_bt(A, xts[0], xt, 0, 5)
        ld_w(A, 0, 0)
        ld_bt(A, xts[0], xt, 0, 6)
        ld_bt(A, xts[0], xt, 0, 7)
        A.dma_start(w8tiles[1], w8a[1])
        ld_w(A, 0, 1)
        ld_w(A, 1, 0)
        ld_bt(A, xts[0], xt, 0, 2)
        ld_bt(A, xts[0], xt, 0, 3)
        ld_w(A, 0, 2)
        ld_w(A, 1, 1)
        ld_w(A, 0, 3)
        ld_w(A, 1, 2)
        ld_w(A, 0, 4)
        ld_w(A, 1, 3)
        ld_w(A, 0, 5)
        ld_w(A, 1, 4)
        ld_w(A, 0, 6)
        ld_w(A, 1, 5)
        ld_w(A, 0, 7)
        ld_w(A, 1, 6)
        ld_w(A, 1, 7)
        for m in (2, 3, 4, 5):
            A.dma_start(w8tiles[m], w8b[m - 2])
        A.dma_start(wtiles[2], wtsb[0])
        A.dma_start(wtiles[3], wtsb[1])
        # ring B: bias, chunk-0 fp8 inputs, h0 (matmul ks first), the two
        # late bf16 mats, then all of chunk 1.
        Bq.dma_start(btile, bias[:, :])
        Bq.dma_start(x8ts[0], x8[0])
        Bq.dma_start(h8ts[0], h8[0])
        for k in (4, 5, 6, 7, 0, 1, 2, 3):
            ld_bt(Bq, hts[0], ht, 0, k)
        Bq.dma_start(wtiles[4], wtsb[2])
        Bq.dma_start(wtiles[5], wtsb[3])
        Bq.dma_start(x8ts[1], x8[1])
        Bq.dma_start(h8ts[1], h8[1])
        for k in (2, 3, 4, 5, 6, 7):
            ld_bt(Bq, xts[1], xt, 1, k)
        for k in range(NK):
            ld_bt(Bq, hts[1], ht, 1, k)

        def w_ap(m, j, k):
            jw = NK - KS[m]
            return wtiles[m][:, (j * jw + (k - KS[m])) * 128 : (j * jw + (k - KS[m]) + 1) * 128]

        def mm_half(ps, m, j, mov, mov8, start, stop):
            """fp8 DoubleRow pairs then bf16 k-tiles into NB psum banks.

            mov: bf16 [128, NK*CW] tile; mov8: fp8 [128, >=2*P8[m], CW]."""
            for q in range(P8[m]):
                lhsT = w8tiles[m][:, 2 * q : 2 * q + 2, j * 128 : (j + 1) * 128]
                for b in range(NB):
                    nc.tensor.matmul(
                        ps[b],
                        lhsT,
                        mov8[:, 2 * q : 2 * q + 2, b * BW : (b + 1) * BW],
                        start=(start and q == 0),
                        stop=False,
                        perf_mode=DR,
                    )
            for k in range(KS[m], NK):
                lhsT = w_ap(m, j, k)
                for b in range(NB):
                    nc.tensor.matmul(
                        ps[b],
                        lhsT,
                        mov[:, k * CW + b * BW : k * CW + (b + 1) * BW],
                        start=False,
                        stop=(stop and k == NK - 1),
                    )

        def activate(dst, ps, fn, bcol):
            for b in range(NB):
                nc.scalar.activation(
                    dst[:, b * BW : (b + 1) * BW], ps[b], fn,
                    bias=btile[:, bcol : bcol + 1],
                )

        for ch in range(N_CHUNKS):
            xc, hc = xts[ch], hts[ch]
            x8c, h8c = x8ts[ch], h8ts[ch]

            hrs = [None] * NH
            hr8 = hr8p.tile([128, 2, CW], FP8, name="hr8tile")

            def finish_r(j, ps):
                mm_half(ps, 1, j, hc, h8c, start=False, stop=True)
                rtile = rp.tile([128, CW], BF16, name="rtile")
                activate(rtile, ps, AF.Sigmoid, j)
                hj = hc[:, j * CW : (j + 1) * CW]
                if j < 2:
                    # U's fp8 slice: write h*r straight to fp8 (pair 0)
                    nc.vector.tensor_mul(hr8[:, j : j + 1, :], hj, rtile)
                else:
                    hrtile = hrp.tile([128, CW], BF16, name="hrtile")
                    nc.vector.tensor_mul(hrtile, hj, rtile)
                    hrs[j] = hrtile

            ps_list = []
            for j in range(NH):
                ps = [pp.tile([128, BW], F32, name="ps") for _ in range(NB)]
                mm_half(ps, 0, j, xc, x8c, start=True, stop=False)
                ps_list.append(ps)
                if j >= 2:
                    finish_r(j - 2, ps_list[j - 2])
            finish_r(NH - 2, ps_list[NH - 2])
            finish_r(NH - 1, ps_list[NH - 1])

            for j in range(NH):
                psu = [pp.tile([128, BW], F32, name="ps") for _ in range(NB)]
                mm_half(psu, 2, j, xc, x8c, start=True, stop=False)
                mm_half(psu, 3, j, hc, h8c, start=False, stop=True)
                util = up.tile([128, CW], BF16, name="utile")
                activate(util, psu, AF.Sigmoid, NH + j)

                psc = [pp.tile([128, BW], F32, name="ps") for _ in range(NB)]
                mm_half(psc, 4, j, xc, x8c, start=True, stop=False)
                # U-side of c: fp8 pair 0 from hr8, bf16 k>=2 from hrs
                lhsT8 = w8tiles[5][:, 0:2, j * 128 : (j + 1) * 128]
                for b in range(NB):
                    nc.tensor.matmul(
                        psc[b], lhsT8, hr8[:, :, b * BW : (b + 1) * BW],
                        start=False, stop=False, perf_mode=DR,
                    )
                for k in range(2, NK):
                    lhsT = w_ap(5, j, k)
                    for b in range(NB):
                        nc.tensor.matmul(
                            psc[b], lhsT, hrs[k][:, b * BW : (b + 1) * BW],
                            start=False, stop=(k == NK - 1),
                        )
                ctile = cp.tile([128, CW], BF16, name="ctile")
                activate(ctile, psc, AF.Tanh, 2 * NH + j)

                hj = hc[:, j * CW : (j + 1) * CW]
                t = op.tile([128, CW], F32, name="ttile")
                for b in range(NB):
                    sl = slice(b * BW, (b + 1) * BW)
                    nc.vector.tensor_sub(t[:, sl], ctile[:, sl], hj[:, sl])
                    nc.vector.tensor_mul(t[:, sl], util[:, sl], t[:, sl])
                    nc.vector.tensor_add(t[:, sl], t[:, sl], hj[:, sl])
                    nc.sync.dma_start(
                        out[
                            j * 128 : (j + 1) * 128,
                            ch * CW + b * BW : ch * CW + (b + 1) * BW,
                        ],
                        t[:, sl],
                    )

    nc.compile()
    return nc


def pack_inputs(inputs):
    x = np.asarray(inputs["x_t"], np.float32)
    h = np.asarray(inputs["h_prev"], np.float32)

    mats = [inputs["W_r"], inputs["U_r"], inputs["W_u"], inputs["U_u"],
            inputs["W"], inputs["U"]]
    wtsa = np.empty((2, 128, NH * (NK - KS[0]) * 128), ml_dtypes.bfloat16)
    wtsb = np.empty((4, 128, NH * (NK - KS[2]) * 128), ml_dtypes.bfloat16)
    w8a = np.empty((2, 128, 2 * P8[0], NH * 128), NPF8)
    w8b = np.empty((4, 128, 2 * P8[2], NH * 128), NPF8)
    for i, m in enumerate(mats):
        mt = np.asarray(m, np.float32).T  # [in(k), out(j)]
        t4 = mt.reshape(NK, 128, NH, 128)  # [k, p, j, c]
        ks = KS[i]
        bf = (
            t4[ks:]
            .transpose(1, 2, 0, 3)  # [p, j, k', c]
            .reshape(128, NH * (NK - ks) * 128)
            .astype(ml_dtypes.bfloat16)
        )
        f8 = (
            t4[:ks]
            .transpose(1, 0, 2, 3)  # [p, kk, j, c]
            .reshape(128, ks, NH * 128)
            .astype(NPF8)
        )
        if i < 2:
            wtsa[i] = bf
            w8a[i] = f8
        else:
            wtsb[i - 2] = bf
            w8b[i - 2] = f8

    b_r = np.asarray(inputs["b_Wr"], np.float32) + np.asarray(inputs["b_Ur"], np.float32)
    b_u = np.asarray(inputs["b_Wu"], np.float32) + np.asarray(inputs["b_Uu"], np.float32)
    b_c = np.asarray(inputs["b_W"], np.float32) + np.asarray(inputs["b_U"], np.float32)
    bias = np.concatenate(
        [bb.reshape(NH, 128).T for bb in (b_r, b_u, b_c)], axis=1
    ).astype(np.float32)

    def pack_bt(a_rows):
        at = np.ascontiguousarray(a_rows.T)  # [D, B_CORE] f32
        o = np.empty((N_CHUNKS, 128, NK * CW), ml_dtypes.bfloat16)
        o8 = np.empty((N_CHUNKS, 128, 4, CW), NPF8)
        for ch in range(N_CHUNKS):
            blk = at[:, ch * CW : (ch + 1) * CW].reshape(NK, 128, CW)
            o[ch] = blk.transpose(1, 0, 2).reshape(128, NK * CW).astype(ml_dtypes.bfloat16)
            o8[ch] = blk[:4].transpose(1, 0, 2).astype(NPF8)
        return o, o8

    in_maps = []
    for c in range(N_CORES):
        rows = slice(c * B_CORE, (c + 1) * B_CORE)
        xb, xf8 = pack_bt(x[rows])
        hb, hf8 = pack_bt(h[rows])
        in_maps.append(
            {
                "xt": xb, "ht": hb, "x8": xf8, "h8": hf8,
                "wtsa": wtsa, "wtsb": wtsb, "w8a": w8a, "w8b": w8b,
                "bias": bias,
            }
        )
    return in_maps


_NC_CACHE = {}


def _get_nc():
    if "nc" not in _NC_CACHE:
        _NC_CACHE["nc"] = build_nc()
    return _NC_CACHE["nc"]


def _run(inputs, **spmd_kwargs):
    nc = _get_nc()
    in_maps = pack_inputs(inputs)
    res = run_bass_kernel_spmd(nc, in_maps, list(range(N_CORES)), **spmd_kwargs)
    out = np.empty((B, D), np.float32)
    for c in range(N_CORES):
        out[c * B_CORE : (c + 1) * B_CORE, :] = res.results[c]["out"].T
    return out, [res]


def kernel(**inputs):
    out, _ = _run(inputs)
    return out


# revision 17
# speedup vs baseline: 1.4642x; 1.0016x over previous
"""GRU cell kernel for Trainium2 — v4: bf16 + partial-fp8 (DoubleRow).

Same structure as kernel.py (single dispatch, 8-core DP, 2 chunks,
transposed space, big demand-ordered DMAs), but a leading slice of each
matmul contraction runs in fp8e4m3 with DoubleRow perf mode (2 fp8
weights per PE cell -> 256-deep contraction per instruction, ~1.8x the
bf16 rate for that slice):

    W_r/U_r: first 512 of 1024 contraction rows in fp8  (p=2 pairs)
    W_u/U_u/W/U: first 256 rows in fp8                  (p=1 pair)

The sigmoid/tanh nonlinearities damp the extra quantization noise;
simulated end-to-end rel err 1.54e-2 vs the 2e-2 gate (pure bf16 is
3.7e-3).  bf16 weight tiles drop their fp8-covered k-slices so
everything still fits in ~196KB/partition of SBUF.
"""

import sys

sys.path.insert(0, "/opt/trn_rl_repo")

import numpy as np
import ml_dtypes
from contextlib import ExitStack

import concourse.bass as bass
import concourse.bacc as bacc
import concourse.mybir as mybir
from concourse import tile
from concourse.bass_utils import run_bass_kernel_spmd

BF16 = mybir.dt.bfloat16
FP8 = mybir.dt.float8e4
F32 = mybir.dt.float32
AF = mybir.ActivationFunctionType
DR = mybir.MatmulPerfMode.DoubleRow
NPF8 = ml_dtypes.float8_e4m3

N_CORES = 8
B = 16384
D = 1024
B_CORE = B // N_CORES
N_CHUNKS = 2
CW = B_CORE // N_CHUNKS  # 1024
BW = 512
NH = D // 128
NK = D // 128
NB = CW // BW
# fp8 pairs per mat: 0=W_r 1=U_r 2=W_u 3=U_u 4=W 5=U
P8 = (4, 4, 1, 1, 1, 1)
KS = tuple(2 * p for p in P8)  # first bf16 k-tile per mat


def build_nc():
    nc = bacc.Bacc("TRN2", target_bir_lowering=False)
    # slim bf16 weights: mat m keeps k-tiles [KS[m], NK); W_r/U_r are
    # fully fp8 so they have no bf16 tensor at all
    wtsb = nc.dram_tensor("wtsb", [4, 128, NH * (NK - KS[2]) * 128], BF16,
                          kind="ExternalInput")
    w8a = nc.dram_tensor("w8a", [2, 128, 2 * P8[0], NH * 128], FP8,
                         kind="ExternalInput")
    w8b = nc.dram_tensor("w8b", [4, 128, 2 * P8[2], NH * 128], FP8,
                         kind="ExternalInput")
    xt = nc.dram_tensor("xt", [N_CHUNKS, 128, NK * CW], BF16, kind="ExternalInput")
    ht = nc.dram_tensor("ht", [N_CHUNKS, 128, NK * CW], BF16, kind="ExternalInput")
    x8 = nc.dram_tensor("x8", [N_CHUNKS, 128, 2 * P8[0], CW], FP8, kind="ExternalInput")
    h8 = nc.dram_tensor("h8", [N_CHUNKS, 128, 2 * P8[0], CW], FP8, kind="ExternalInput")
    bias = nc.dram_tensor("bias", [128, 3 * NH], F32, kind="ExternalInput")
    out = nc.dram_tensor("out", [D, B_CORE], F32, kind="ExternalOutput")

    with tile.TileContext(nc) as tc, ExitStack() as ctx:
        wp = ctx.enter_context(tc.tile_pool(name="wp", bufs=1))
        w8p = ctx.enter_context(tc.tile_pool(name="w8p", bufs=1))
        xp = ctx.enter_context(tc.tile_pool(name="xp", bufs=N_CHUNKS))
        hp = ctx.enter_context(tc.tile_pool(name="hp", bufs=N_CHUNKS))
        x8p = ctx.enter_context(tc.tile_pool(name="x8p", bufs=N_CHUNKS))
        h8p = ctx.enter_context(tc.tile_pool(name="h8p", bufs=N_CHUNKS))
        hrp = ctx.enter_context(tc.tile_pool(name="hrp", bufs=NH - 2))
        hr8p = ctx.enter_context(tc.tile_pool(name="hr8p", bufs=2))
        rp = ctx.enter_context(tc.tile_pool(name="rp", bufs=2))
        up = ctx.enter_context(tc.tile_pool(name="up", bufs=2))
        cp = ctx.enter_context(tc.tile_pool(name="cp", bufs=2))
        op = ctx.enter_context(tc.tile_pool(name="op", bufs=2))
        bp = ctx.enter_context(tc.tile_pool(name="bp", bufs=1))
        pp = ctx.enter_context(tc.tile_pool(name="pp", bufs=8, space="PSUM"))

        # tiles (distinct tag per mat via bufs=1 + unique names is fine here:
        # each is a singleton)
        wtiles = [None, None]
        for m in range(2, 6):
            jw = (NK - KS[m]) * 128
            wtiles.append(wp.tile([128, NH * jw], BF16, name=f"w{m}", bufs=1))
        w8tiles = []
        for m in range(6):
            w8tiles.append(
                w8p.tile([128, 2 * P8[m], NH * 128], FP8, name=f"w8_{m}", bufs=1)
            )
        xts, hts, x8ts, h8ts = [], [], [], []
        for ch in range(N_CHUNKS):
            xts.append(xp.tile([128, NK * CW], BF16, name="xtile"))
            hts.append(hp.tile([128, NK * CW], BF16, name="htile"))
            x8ts.append(x8p.tile([128, 2 * P8[0], CW], FP8, name="x8tile"))
            h8ts.append(h8p.tile([128, 2 * P8[0], CW], FP8, name="h8tile"))
        btile = bp.tile([128, 3 * NH], F32, name="btile")

        def ld_w(eng, m, j):
            jw = (NK - KS[m]) * 128
            eng.dma_start(
                wtiles[m][:, j * jw : (j + 1) * jw],
                wtsb[m - 2, :, j * jw : (j + 1) * jw],
            )

        def ld_bt(eng, tile_, dram, ch, k):
            eng.dma_start(
                tile_[:, k * CW : (k + 1) * CW], dram[ch, :, k * CW : (k + 1) * CW]
            )

        A, Bq = nc.scalar, nc.sync
        # (4,1,1) demand schedule.  The R phase is now pure fp8 (w8a + x8 +
        # h8 = 4MB, no bf16 x/h) and runs ~3.5us/j, so both the R inputs and
        # the UC-phase weights are on tight deadlines: the big fp8 blocks
        # are half-split so the j0/fr(j0) pipeline starts on pairs 0-1
        # while pairs 2-3 are still in flight, and the four UC bf16 mats
        # are j-pair-sliced, interleaved in first-use order across both
        # rings (m2/m3 on A, m4/m5 on B).
        def ld_half(eng, tile_, dram_sl, lo, hi):
            eng.dma_start(tile_[:, lo:hi, :], dram_sl[:, lo:hi, :])

        np8 = 2 * P8[0]  # 8 fp8 k-tiles
        # ring A: R-phase fp8 weights, then UC fp8 weights, then m2/m3 bf16
        ld_half(A, w8tiles[0], w8a[0], 0, np8 // 2)
        ld_half(A, w8tiles[0], w8a[0], np8 // 2, np8)
        ld_half(A, w8tiles[1], w8a[1], 0, np8 // 2)
        ld_half(A, w8tiles[1], w8a[1], np8 // 2, np8)
        for m in (2, 3, 4, 5):
            A.dma_start(w8tiles[m], w8b[m - 2])
        for jj in (0, 2, 4, 6):
            for m in (2, 3):
                ld_w(A, m, jj)
                ld_w(A, m, jj + 1)
        # ring B: bias, R-phase fp8 inputs (half-split), h0, x0 (bf16 ks
        # only), m4/m5 bf16 j-pair-sliced, then all of chunk 1.
        Bq.dma_start(btile, bias[:, :])
        ld_half(Bq, x8ts[0], x8[0], 0, np8 // 2)
        ld_half(Bq, x8ts[0], x8[0], np8 // 2, np8)
        ld_half(Bq, h8ts[0], h8[0], 0, np8 // 2)
        ld_half(Bq, h8ts[0], h8[0], np8 // 2, np8)
        for k in (0, 1, 2, 3, 4, 5, 6, 7):
            ld_bt(Bq, hts[0], ht, 0, k)
        for k in (2, 3, 4, 5, 6, 7):
            ld_bt(Bq, xts[0], xt, 0, k)
        for jj in (0, 2, 4, 6):
            for m in (4, 5):
                ld_w(Bq, m, jj)
                ld_w(Bq, m, jj + 1)
        Bq.dma_start(x8ts[1], x8[1])
        Bq.dma_start(h8ts[1], h8[1])
        for k in (2, 3, 4, 5, 6, 7):
            ld_bt(Bq, xts[1], xt, 1, k)
        for k in range(NK):
            ld_bt(Bq, hts[1], ht, 1, k)

        def w_ap(m, j, k):
            jw = NK - KS[m]
            return wtiles[m][:, (j * jw + (k - KS[m])) * 128 : (j * jw + (k - KS[m]) + 1) * 128]

        def mm_half(ps, m, j, mov, mov8, start, stop):
            """fp8 DoubleRow pairs then bf16 k-tiles into NB psum banks.

            mov: bf16 [128, NK*CW] tile; mov8: fp8 [128, >=2*P8[m], CW]."""
            for q in range(P8[m]):
                lhsT = w8tiles[m][:, 2 * q : 2 * q + 2, j * 128 : (j + 1) * 128]
                for b in range(NB):
                    nc.tensor.matmul(
                        ps[b],
                        lhsT,
                        mov8[:, 2 * q : 2 * q + 2, b * BW : (b + 1) * BW],
                        start=(start and q == 0),
                        stop=False,
                        perf_mode=DR,
                    )
            for k in range(KS[m], NK):
                lhsT = w_ap(m, j, k)
                for b in range(NB):
                    nc.tensor.matmul(
                        ps[b],
                        lhsT,
                        mov[:, k * CW + b * BW : k * CW + (b + 1) * BW],
                        start=False,
                        stop=(stop and k == NK - 1),
                    )

        def activate(dst, ps, fn, bcol):
            for b in range(NB):
                nc.scalar.activation(
                    dst[:, b * BW : (b + 1) * BW], ps[b], fn,
                    bias=btile[:, bcol : bcol + 1],
                )

        for ch in range(N_CHUNKS):
            xc, hc = xts[ch], hts[ch]
            x8c, h8c = x8ts[ch], h8ts[ch]

            hrs = [None] * NH
            hr8 = hr8p.tile([128, 2, CW], FP8, name="hr8tile")

            def finish_r(j, ps):
                mm_half(ps, 1, j, hc, h8c, start=False, stop=True)
                rtile = rp.tile([128, CW], BF16, name="rtile")
                activate(rtile, ps, AF.Sigmoid, j)
                hj = hc[:, j * CW : (j + 1) * CW]
                if j < 2:
                    # U's fp8 slice: write h*r straight to fp8 (pair 0)
                    nc.vector.tensor_mul(hr8[:, j : j + 1, :], hj, rtile)
                else:
                    hrtile = hrp.tile([128, CW], BF16, name="hrtile")
                    nc.vector.tensor_mul(hrtile, hj, rtile)
                    hrs[j] = hrtile

            ps_list = []
            for j in range(NH):
                ps = [pp.tile([128, BW], F32, name="ps") for _ in range(NB)]
                mm_half(ps, 0, j, xc, x8c, start=True, stop=False)
                ps_list.append(ps)
                if j >= 2:
                    finish_r(j - 2, ps_list[j - 2])
            finish_r(NH - 2, ps_list[NH - 2])
            finish_r(NH - 1, ps_list[NH - 1])

            for j in range(NH):
                psu = [pp.tile([128, BW], F32, name="ps") for _ in range(NB)]
                mm_half(psu, 2, j, xc, x8c, start=True, stop=False)
                mm_half(psu, 3, j, hc, h8c, start=False, stop=True)
                util = up.tile([128, CW], BF16, name="utile")
                activate(util, psu, AF.Sigmoid, NH + j)

                psc = [pp.tile([128, BW], F32, name="ps") for _ in range(NB)]
                mm_half(psc, 4, j, xc, x8c, start=True, stop=False)
                # U-side of c: fp8 pair 0 from hr8, bf16 k>=2 from hrs
                lhsT8 = w8tiles[5][:, 0:2, j * 128 : (j + 1) * 128]
                for b in range(NB):
                    nc.tensor.matmul(
                        psc[b], lhsT8, hr8[:, :, b * BW : (b + 1) * BW],
                        start=False, stop=False, perf_mode=DR,
                    )
                for k in range(2, NK):
                    lhsT = w_ap(5, j, k)
                    for b in range(NB):
                        nc.tensor.matmul(
                            psc[b], lhsT, hrs[k][:, b * BW : (b + 1) * BW],
                            start=False, stop=(k == NK - 1),
                        )
                ctile = cp.tile([128, CW], BF16, name="ctile")
                activate(ctile, psc, AF.Tanh, 2 * NH + j)

                hj = hc[:, j * CW : (j + 1) * CW]
                t = op.tile([128, CW], F32, name="ttile")
                for b in range(NB):
                    sl = slice(b * BW, (b + 1) * BW)
                    nc.vector.tensor_sub(t[:, sl], ctile[:, sl], hj[:, sl])
                    nc.vector.tensor_mul(t[:, sl], util[:, sl], t[:, sl])
                    nc.vector.tensor_add(t[:, sl], t[:, sl], hj[:, sl])
                    nc.sync.dma_start(
                        out[
                            j * 128 : (j + 1) * 128,
                            ch * CW + b * BW : ch * CW + (b + 1) * BW,
                        ],
                        t[:, sl],
                    )

    nc.compile()
    return nc


def pack_inputs(inputs):
    x = np.asarray(inputs["x_t"], np.float32)
    h = np.asarray(inputs["h_prev"], np.float32)

    mats = [inputs["W_r"], inputs["U_r"], inputs["W_u"], inputs["U_u"],
            inputs["W"], inputs["U"]]
    wtsb = np.empty((4, 128, NH * (NK - KS[2]) * 128), ml_dtypes.bfloat16)
    w8a = np.empty((2, 128, 2 * P8[0], NH * 128), NPF8)
    w8b = np.empty((4, 128, 2 * P8[2], NH * 128), NPF8)
    for i, m in enumerate(mats):
        mt = np.asarray(m, np.float32).T  # [in(k), out(j)]
        t4 = mt.reshape(NK, 128, NH, 128)  # [k, p, j, c]
        ks = KS[i]
        bf = (
            t4[ks:]
            .transpose(1, 2, 0, 3)  # [p, j, k', c]
            .reshape(128, NH * (NK - ks) * 128)
            .astype(ml_dtypes.bfloat16)
        )
        f8 = (
            t4[:ks]
            .transpose(1, 0, 2, 3)  # [p, kk, j, c]
            .reshape(128, ks, NH * 128)
            .astype(NPF8)
        )
        if i < 2:
            w8a[i] = f8
        else:
            wtsb[i - 2] = bf
            w8b[i - 2] = f8

    b_r = np.asarray(inputs["b_Wr"], np.float32) + np.asarray(inputs["b_Ur"], np.float32)
    b_u = np.asarray(inputs["b_Wu"], np.float32) + np.asarray(inputs["b_Uu"], np.float32)
    b_c = np.asarray(inputs["b_W"], np.float32) + np.asarray(inputs["b_U"], np.float32)
    bias = np.concatenate(
        [bb.reshape(NH, 128).T for bb in (b_r, b_u, b_c)], axis=1
    ).astype(np.float32)

    def pack_bt(a_rows):
        at = np.ascontiguousarray(a_rows.T)  # [D, B_CORE] f32
        o = np.empty((N_CHUNKS, 128, NK * CW), ml_dtypes.bfloat16)
        o8 = np.empty((N_CHUNKS, 128, 2 * P8[0], CW), NPF8)
        for ch in range(N_CHUNKS):
            blk = at[:, ch * CW : (ch + 1) * CW].reshape(NK, 128, CW)
            o[ch] = blk.transpose(1, 0, 2).reshape(128, NK * CW).astype(ml_dtypes.bfloat16)
            o8[ch] = blk[: 2 * P8[0]].transpose(1, 0, 2).astype(NPF8)
        return o, o8

    in_maps = []
    for c in range(N_CORES):
        rows = slice(c * B_CORE, (c + 1) * B_CORE)
        xb, xf8 = pack_bt(x[rows])
        hb, hf8 = pack_bt(h[rows])
        in_maps.append(
            {
                "xt": xb, "ht": hb, "x8": xf8, "h8": hf8,
                "wtsb": wtsb, "w8a": w8a, "w8b": w8b,
                "bias": bias,
            }
        )
    return in_maps


_NC_CACHE = {}


def _get_nc():
    if "nc" not in _NC_CACHE:
        _NC_CACHE["nc"] = build_nc()
    return _NC_CACHE["nc"]


def _run(inputs, **spmd_kwargs):
    nc = _get_nc()
    in_maps = pack_inputs(inputs)
    res = run_bass_kernel_spmd(nc, in_maps, list(range(N_CORES)), **spmd_kwargs)
    out = np.empty((B, D), np.float32)
    for c in range(N_CORES):
        out[c * B_CORE : (c + 1) * B_CORE, :] = res.results[c]["out"].T
    return out, [res]


def kernel(**inputs):
    out, _ = _run(inputs)
    return out
